# revision 44
# baseline (speedup 1.0000x reference)
"""Multi-head attention (RoPE + doc-masked causal) on 8 Trainium2 cores.

Sharding: tensor-parallel over heads. H=16 heads -> 2 heads/core.
Each core computes q/k/v projections for its head group (Wq/Wk/Wv column
slices), attention for its 2 heads, and a partial output projection
(Wo row slice). Host sums the 8 partial outputs.

Layout strategy (per core):
  - qT/kT [hd=128, t] computed directly by projection matmuls (lhsT=W chunk,
    rhs=xT chunk); RoPE applied in this layout on DVE with partition-shifted
    PSUM reads.
  - scoresT [s, t] = k @ qT via matmul(lhsT=kT_block, rhs=qT_chunk); exp on
    ACT (scale=1/sqrt(hd) folded in); softmax denominator via ones-matmul;
    PV as outT[hd, t] = v.T @ pT; normalization by 1/l broadcast across
    partitions with a K=1 matmul; final y = outT_scaled.T @ Wo rows.
  - Matmuls in bf16 (fp32 PSUM accumulation; pT rounding cancels in the
    softmax ratio). The 1/l normalization chain stays fp32/fp32r.
  - doc_ids are sorted -> allowed region of scoresT is block-diagonal AND
    causal. The program is specialized at build time: fully-masked
    128x512 tiles are skipped entirely (matmuls included), fully-allowed
    tiles skip masking, boundary tiles get a host-precomputed bf16 0/1
    mask multiply.

Pipeline:
  - xT is preloaded whole into SBUF ([128, chunk, k, 512] layout relaid on
    the host for contiguous 2-8KB/partition DMA lines); DMA order on the
    sync/gpsimd queues tracks PE consumption (cos/sin first, wq|xt0 on
    sync, wk|xt0 on gpsimd, then per-chunk masks / wv / wo / xt1-3).
  - Per chunk, the exp-paced QK wave (score banks drain at ACT exp rate
    ~550ns/tile) is interleaved into the v-projection's dense matmul
    stream; denominators are folded into the wave and each head's
    reciprocal + 1/l broadcast chain completes under the PV stream, so
    the post-PV critical path is a single STT per head.
  - y casts alternate DVE/ACT; y streams out as 256KB dc-pairs on the two
    HWDGE queues (sync/scalar) with 4 PSUM banks (pB/pD alternating).
    gpsimd issues nothing after the input loads so its fixed ~8us
    software-DGE drain overlaps the kernel body instead of the tail.
"""
import sys

sys.path.insert(0, "/opt/trn_rl_repo")

import numpy as np

import concourse.bacc as bacc
from concourse import bass_utils, mybir
from concourse.tile import TileContext

dt = mybir.dt

B, T, D, H, HD = 1, 2048, 2048, 16, 128
NCORES = 8
HPC = H // NCORES          # heads per core = 2
J = HPC * HD               # head-group width per core = 256
TCH = 512                  # t-chunk (PSUM bank = 512 fp32)
NTCH = T // TCH            # 4
KC = D // 128              # 16 contraction chunks
NTB = TCH // 128           # t-blocks per chunk = 4
SCALE = 1.0 / np.sqrt(HD)
WPIECE = 2                 # weight DMA split granularity (k-chunks per DMA)


def _plan(doc):
    """Per t-chunk: list of (s_block, mask_idx|None); masks: list of [128,512]."""
    doc = np.asarray(doc).astype(np.int64)
    is_sorted = bool(np.all(doc[1:] >= doc[:-1]))
    plans, masks = [], []
    for ic in range(NTCH):
        t0 = ic * TCH
        tcols = np.arange(t0, t0 + TCH)
        if is_sorted:
            s_lo = int(np.searchsorted(doc, doc[t0], side="left"))
        else:
            s_lo = 0  # scan all blocks; allowed.any() filter keeps correctness
        blocks = []
        for b in range(s_lo // 128, (t0 + TCH - 1) // 128 + 1):
            srows = np.arange(b * 128, b * 128 + 128)
            allowed = (srows[:, None] <= tcols[None, :]) & (
                doc[srows][:, None] == doc[tcols][None, :]
            )
            if not allowed.any():
                continue
            cols = np.flatnonzero(allowed.any(axis=0))
            c0, c1 = int(cols[0]), int(cols[-1]) + 1
            if allowed[:, c0:c1].all():
                blocks.append((b, None, c0, c1))
            else:
                masks.append(allowed.astype(np.float32))
                blocks.append((b, len(masks) - 1, c0, c1))
        plans.append(blocks)
    return plans, masks


def _build(plans, n_masks):
    nc = bacc.Bacc("TRN2", target_bir_lowering=False, debug=False)
    f32, f32r, bf16 = dt.float32, dt.float32r, dt.bfloat16

    # xT relaid host-side to [128, chunk, k, 512] so chunk slabs are
    # contiguous per partition
    xtd = nc.dram_tensor("xtd", [128, NTCH, KC, TCH], bf16, kind="ExternalInput").ap()
    wq = nc.dram_tensor("wq", [128, KC, HPC, 128], bf16, kind="ExternalInput").ap()
    wk = nc.dram_tensor("wk", [128, KC, HPC, 128], bf16, kind="ExternalInput").ap()
    wv = nc.dram_tensor("wv", [128, KC, J], bf16, kind="ExternalInput").ap()
    wo = nc.dram_tensor("wo", [128, HPC, D], bf16, kind="ExternalInput").ap()
    cosT = nc.dram_tensor("cosT", [HD, T], bf16, kind="ExternalInput").ap()
    sinT = nc.dram_tensor("sinT", [HD, T], bf16, kind="ExternalInput").ap()
    onesb_d = nc.dram_tensor("onesb", [128, 1], bf16, kind="ExternalInput").ap()
    mk = nc.dram_tensor(
        "masks", [max(1, n_masks), 128, TCH], bf16, kind="ExternalInput"
    ).ap()
    y = nc.dram_tensor("y", [T, D], bf16, kind="ExternalOutput").ap()

    MULT = mybir.AluOpType.mult
    EXP = mybir.ActivationFunctionType.Exp

    with TileContext(nc) as tc:
        with (
            tc.tile_pool(name="consts", bufs=1) as consts,
            tc.tile_pool(name="rope", bufs=4) as ropep,
            tc.tile_pool(name="ptp", bufs=16) as ptp,
            tc.tile_pool(name="smallp", bufs=4) as smallp,
            tc.tile_pool(name="yp", bufs=8) as yp,
            tc.tile_pool(name="ps", bufs=1, space="PSUM") as ps,
        ):
            # ---- static SBUF tensors ----
            xt_sb = consts.tile([128, NTCH, KC, TCH], bf16)
            wq_sb = consts.tile([128, KC, HPC, 128], bf16)
            wk_sb = consts.tile([128, KC, HPC, 128], bf16)
            wv_sb = consts.tile([128, KC, J], bf16)
            wo_sb = consts.tile([128, HPC, D], bf16)
            cos_sb = consts.tile([HD, T], bf16)
            sin_sb = consts.tile([HD, T], bf16)
            ones_bf = consts.tile([128, 1], bf16)
            ones_row = consts.tile([1, 128], bf16)
            nc.vector.memset(ones_row, 1.0)
            krope_sb = consts.tile([HD, HPC, T], bf16)
            v_sb = consts.tile([128, KC, J], bf16)
            mk_tiles = [
                consts.tile([128, TCH], bf16, name=f"mkt_{mi}")
                for mi in range(n_masks)
            ]

            # ---- DMA issue order tracks PE consumption order ----
            # sync queue: wq pieces interleaved with even xt0 pieces, wo,
            #             first halves of xt1-3
            # gpsimd queue: cos/sin, wk pieces + odd xt0 pieces, chunk-0
            #             masks, wv, ones, remaining masks + xt1-3 halves
            # chunk-0 RoPE only reads cos/sin cols 0:512 — load just those
            # upfront (2x128KB) so the xt0 pieces behind them on the gpsimd
            # queue arrive ~7us earlier; the rest follows after wv
            nc.gpsimd.dma_start(out=cos_sb[:, 0:TCH], in_=cosT[:, 0:TCH])
            nc.gpsimd.dma_start(out=sin_sb[:, 0:TCH], in_=sinT[:, 0:TCH])
            for p in range(KC // WPIECE):
                ksl = slice(p * WPIECE, (p + 1) * WPIECE)
                nc.sync.dma_start(out=wq_sb[:, ksl], in_=wq[:, ksl])
                nc.gpsimd.dma_start(out=wk_sb[:, ksl], in_=wk[:, ksl])
                eng = nc.sync if p % 2 == 0 else nc.gpsimd
                eng.dma_start(out=xt_sb[:, 0, ksl], in_=xtd[:, 0, ksl])

            # masks needed by attention chunk ic; chunk 0's first
            masks_by_chunk = [
                [mi for _, mi, _, _ in plans[ic] if mi is not None]
                for ic in range(NTCH)
            ]
            # chunk-0 masks go on sync (its wo deadline is relaxed by the
            # y-deferral), pulling wv ~1.5us earlier on the gpsimd pole
            for mi in masks_by_chunk[0]:
                nc.sync.dma_start(out=mk_tiles[mi], in_=mk[mi])
            for p in range(4):
                ksl = slice(p * 4, (p + 1) * 4)
                nc.gpsimd.dma_start(out=wv_sb[:, ksl], in_=wv[:, ksl])
            nc.gpsimd.dma_start(out=cos_sb[:, TCH : 2 * TCH], in_=cosT[:, TCH : 2 * TCH])
            nc.gpsimd.dma_start(out=sin_sb[:, TCH : 2 * TCH], in_=sinT[:, TCH : 2 * TCH])
            for p in range(4):
                dsl = slice(p * TCH, (p + 1) * TCH)
                nc.sync.dma_start(out=wo_sb[:, :, dsl], in_=wo[:, :, dsl])
            nc.gpsimd.dma_start(out=ones_bf, in_=onesb_d)
            for ic in range(1, NTCH):
                nc.sync.dma_start(out=xt_sb[:, ic, 0:8], in_=xtd[:, ic, 0:8])
                nc.gpsimd.dma_start(out=xt_sb[:, ic, 8:16], in_=xtd[:, ic, 8:16])
                for mi in masks_by_chunk[ic]:
                    nc.gpsimd.dma_start(out=mk_tiles[mi], in_=mk[mi])
                if ic == 1:
                    nc.gpsimd.dma_start(
                        out=cos_sb[:, 2 * TCH : T], in_=cosT[:, 2 * TCH : T]
                    )
                    nc.gpsimd.dma_start(
                        out=sin_sb[:, 2 * TCH : T], in_=sinT[:, 2 * TCH : T]
                    )

            # ---- y output pair: 2 accumulating matmuls + DVE/ACT casts into
            # a 256KB dc-pair, streamed on the HWDGE queues (sync/scalar).
            # gpsimd issues NOTHING after the input loads so its fixed ~8us
            # software-DGE drain runs mid-kernel instead of gating the tail.
            # y PSUM alternates pB/pD tags: 4 banks (d tiles long released).
            def emit_y_pair(ic_y, t0_y, outT, u):
                tb, dc2 = u // 2, u % 2
                trow = t0_y + tb * 128
                y_sb = yp.tile(
                    [128, 2 * TCH], bf16, tag="y", bufs=6, name=f"y_{ic_y}_{u}"
                )
                for half in range(2):
                    dc = dc2 * 2 + half
                    y_ps = ps.tile(
                        [128, TCH], f32, tag="pB" if dc % 2 == 0 else "pD",
                        bufs=2, name=f"yps_{ic_y}_{tb}_{dc}",
                    )
                    for h in range(HPC):
                        nc.tensor.matmul(
                            y_ps,
                            outT[:, h, tb * 128 : (tb + 1) * 128],
                            wo_sb[:, h, dc * TCH : (dc + 1) * TCH],
                            start=(h == 0),
                            stop=(h == HPC - 1),
                        )
                    ysl = y_sb[:, half * TCH : (half + 1) * TCH]
                    if dc % 2 == 0:
                        nc.vector.tensor_copy(ysl, y_ps)
                    else:
                        nc.scalar.copy(ysl, y_ps)
                eng = nc.sync if (tb * 2 + dc2) % 2 == 0 else nc.scalar
                eng.dma_start(
                    out=y[trow : trow + 128, dc2 * 2 * TCH : (dc2 + 1) * 2 * TCH],
                    in_=y_sb,
                )

            prev_y = None
            for ic in range(NTCH):
                t0 = ic * TCH
                tsl = slice(t0, t0 + TCH)

                # ---- q/k projections + RoPE, per j-block; the PREVIOUS
                # chunk's y pairs interleave into this dense stream so the
                # bc/STT chains and y casts never pace the PE ----
                qrope_sb = ropep.tile(
                    [HD, HPC, TCH], bf16, tag="qr", bufs=2, name=f"qr_{ic}"
                )
                for gi, (w_sb, dname, jb) in enumerate(
                    [(wq_sb, "q", 0), (wq_sb, "q", 1), (wk_sb, "k", 0), (wk_sb, "k", 1)]
                ):
                    qk_ps = ps.tile(
                        [128, TCH], f32, tag="pA", bufs=2,
                        name=f"qkps_{dname}_{ic}_{jb}",
                    )
                    for k in range(KC):
                        nc.tensor.matmul(
                            qk_ps,
                            w_sb[:, k, jb, :],
                            xt_sb[:, ic, k, :],
                            start=(k == 0),
                            stop=(k == KC - 1),
                        )
                    # RoPE: out = u*cos + rot(u)*sin; rot = [-u2, u1]
                    csl = cos_sb[:, tsl]
                    ssl = sin_sb[:, tsl]
                    t1 = ropep.tile([HD, TCH], f32, tag="t1", name=f"t1_{dname}_{ic}_{jb}")
                    nc.vector.scalar_tensor_tensor(
                        out=t1, in0=qk_ps, scalar=1.0, in1=csl,
                        op0=MULT, op1=MULT,
                    )
                    t2 = ropep.tile([HD, TCH], f32, tag="t2", name=f"t2_{dname}_{ic}_{jb}")
                    nc.vector.scalar_tensor_tensor(
                        out=t2[0:64, :], in0=qk_ps[64:128, :], scalar=-1.0,
                        in1=ssl[0:64, :], op0=MULT, op1=MULT,
                    )
                    nc.vector.scalar_tensor_tensor(
                        out=t2[64:128, :], in0=qk_ps[0:64, :], scalar=1.0,
                        in1=ssl[64:128, :], op0=MULT, op1=MULT,
                    )
                    if dname == "q":
                        nc.vector.tensor_add(qrope_sb[:, jb, :], t1, t2)
                    else:
                        nc.vector.tensor_add(krope_sb[:, jb, tsl], t1, t2)
                    if prev_y is not None:
                        emit_y_pair(*prev_y, 2 * gi)
                        emit_y_pair(*prev_y, 2 * gi + 1)

                # ---- v projection interleaved with the attention QK+exp
                # wave. The QK issue rate is paced by ACT exp draining the
                # score banks (~520ns/tile vs 216ns matmul), so the v-proj
                # matmuls (7us dense) hide the whole exp-paced QK phase. ----
                nblk = len(plans[ic])
                qk_units = [(h, i) for h in range(HPC) for i in range(nblk)]
                nqk = len(qk_units)
                pts = {}

                def emit_qk(h, i):
                    b, mi, c0, c1 = plans[ic][i]
                    sc_ps = ps.tile(
                        [128, TCH], f32, tag="pS", bufs=2,
                        name=f"scps_{ic}_{h}_{b}",
                    )
                    nc.tensor.matmul(
                        sc_ps[:, c0:c1],
                        krope_sb[:, h, b * 128 : (b + 1) * 128],
                        qrope_sb[:, h, c0:c1],
                        start=True,
                        stop=True,
                    )
                    pt = ptp.tile([128, TCH], bf16, tag="pt", name=f"pt_{ic}_{h}_{b}")
                    nc.scalar.activation(
                        pt[:, c0:c1], sc_ps[:, c0:c1], EXP, bias=0.0, scale=SCALE
                    )
                    if mi is not None:
                        nc.vector.tensor_tensor(
                            out=pt[:, c0:c1], in0=pt[:, c0:c1],
                            in1=mk_tiles[mi][:, c0:c1], op=MULT,
                        )
                    pts[(h, i)] = pt

                d_tiles = [
                    ps.tile([1, TCH], f32, tag="pD", bufs=2, name=f"dps_{ic}_{h}")
                    for h in range(HPC)
                ]

                def emit_dens(h):
                    for i, (b, mi, c0, c1) in enumerate(plans[ic]):
                        nc.tensor.matmul(
                            d_tiles[h][:, c0:c1],
                            ones_bf,
                            pts[(h, i)][:, c0:c1],
                            start=(i == 0),
                            stop=(i == nblk - 1),
                        )

                # 1/l chain: recip + bf16 cast stay on DVE (no engine hop),
                # started inside the wave as soon as each head's dens finish
                def recip_chain(h):
                    rec_f = smallp.tile(
                        [1, TCH], f32, tag="drf", bufs=2, name=f"drf_{ic}_{h}"
                    )
                    nc.vector.reciprocal_approx_fast(out=rec_f, in_=d_tiles[h])
                    rec_b = smallp.tile(
                        [1, TCH], bf16, tag="dr", bufs=2, name=f"dr_{ic}_{h}"
                    )
                    nc.vector.tensor_copy(rec_b, rec_f)
                    return rec_b

                emit_qk(*qk_units[0])
                emit_qk(*qk_units[1])
                nq = 2
                for tb in range(NTB):
                    v_ps = ps.tile([128, J], f32, tag="pB", bufs=2, name=f"vps_{ic}_{tb}")
                    for k in range(KC):
                        nc.tensor.matmul(
                            v_ps,
                            xt_sb[:, ic, k, tb * 128 : (tb + 1) * 128],
                            wv_sb[:, k, :],
                            start=(k == 0),
                            stop=(k == KC - 1),
                        )
                    nc.vector.tensor_copy(v_sb[:, ic * NTB + tb, :], v_ps)
                    # spread remaining QK units across the v t-blocks
                    ntarget = 2 + ((nqk - 2) * (tb + 1) + NTB - 1) // NTB
                    while nq < min(ntarget, nqk):
                        emit_qk(*qk_units[nq])
                        nq += 1
                    if tb == 2:
                        # h0's denominators inside the wave: its pts are done,
                        # and the reciprocal chain hides under tb3 + the PVs
                        emit_dens(0)
                        rb0 = recip_chain(0)

                emit_dens(1)
                rb1 = recip_chain(1)

                def bc_chain(rec_b, h):
                    bc_ps = ps.tile(
                        [128, TCH], f32, tag="pS", bufs=2, name=f"bcps_{ic}_{h}"
                    )
                    nc.tensor.matmul(bc_ps, ones_row, rec_b, start=True, stop=True)
                    bc_sb = smallp.tile(
                        [128, TCH], f32, tag="bcs", bufs=2, name=f"bcs_{ic}_{h}"
                    )
                    nc.scalar.copy(bc_sb, bc_ps)
                    return bc_sb

                def emit_pv(h, bc_sb):
                    o_ps = ps.tile([HD, TCH], f32, tag="pA", bufs=2, name=f"ops_{ic}_{h}")
                    for i, (b, mi, c0, c1) in enumerate(plans[ic]):
                        nc.tensor.matmul(
                            o_ps[:, c0:c1],
                            v_sb[:, b, h * HD : (h + 1) * HD],
                            pts[(h, i)][:, c0:c1],
                            start=(i == 0),
                            stop=(i == nblk - 1),
                        )
                    nc.vector.scalar_tensor_tensor(
                        out=outT_sb[:, h, :], in0=o_ps, scalar=1.0, in1=bc_sb,
                        op0=MULT, op1=MULT,
                    )

                outT_sb = smallp.tile(
                    [HD, HPC, TCH], bf16, tag="outT", bufs=2, name=f"outT_{ic}"
                )
                bc0 = bc_chain(rb0, 0)
                bc1 = bc_chain(rb1, 1)
                emit_pv(0, bc0)
                emit_pv(1, bc1)

                if ic == NTCH - 1:
                    # last chunk: no next projection stream to hide in
                    for u in range(2 * NTB):
                        emit_y_pair(ic, t0, outT_sb, u)
                else:
                    prev_y = (ic, t0, outT_sb)

    nc.compile()
    return nc


_CACHE = {}
_LAST_RESULTS = None


def _get_program(doc):
    key = doc.tobytes()
    if key not in _CACHE:
        plans, masks = _plan(doc)
        nc = _build(plans, len(masks))
        _CACHE[key] = (nc, masks)
    return _CACHE[key]


def kernel(x, Wq, Wk, Wv, Wo, sin, cos, doc_ids, **kwargs):
    import ml_dtypes

    bf = ml_dtypes.bfloat16
    x = np.asarray(x, dtype=np.float32)
    sin = np.asarray(sin, dtype=np.float32)
    cos = np.asarray(cos, dtype=np.float32)
    doc = np.asarray(doc_ids, dtype=np.int32).reshape(-1)

    nc, masks = _get_program(doc)

    # xT [D, T] -> [128, chunk, k, 512]: contiguous 2-8KB/partition DMA lines
    xT = x.reshape(T, D).T.astype(bf)                       # [D, T]
    xtd = np.ascontiguousarray(
        xT.reshape(KC, 128, NTCH, TCH).transpose(1, 2, 0, 3)
    )
    Wq = np.asarray(Wq, dtype=np.float32).astype(bf)
    Wk = np.asarray(Wk, dtype=np.float32).astype(bf)
    Wv = np.asarray(Wv, dtype=np.float32).astype(bf)
    Wo = np.asarray(Wo, dtype=np.float32).astype(bf)
    cosT = np.ascontiguousarray(cos.T).astype(bf)
    sinT = np.ascontiguousarray(sin.T).astype(bf)
    onesb = np.ones((128, 1), bf)
    mk = (
        np.ascontiguousarray(np.stack(masks)).astype(bf)
        if masks
        else np.zeros((1, 128, TCH), bf)
    )

    in_maps = []
    for c in range(NCORES):
        jsl = slice(c * J, (c + 1) * J)
        wq_c = Wq[:, jsl].reshape(KC, 128, HPC, 128).transpose(1, 0, 2, 3)
        wk_c = Wk[:, jsl].reshape(KC, 128, HPC, 128).transpose(1, 0, 2, 3)
        wv_c = Wv[:, jsl].reshape(KC, 128, J).transpose(1, 0, 2)
        wo_c = Wo[jsl, :].reshape(HPC, 128, D).transpose(1, 0, 2)
        in_maps.append(
            {
                "xtd": xtd,
                "wq": np.ascontiguousarray(wq_c),
                "wk": np.ascontiguousarray(wk_c),
                "wv": np.ascontiguousarray(wv_c),
                "wo": np.ascontiguousarray(wo_c),
                "cosT": cosT,
                "sinT": sinT,
                "onesb": onesb,
                "masks": mk,
            }
        )

    res = bass_utils.run_bass_kernel_spmd(
        nc, in_maps, core_ids=list(range(NCORES)), **kwargs
    )
    global _LAST_RESULTS
    _LAST_RESULTS = res
    y = np.zeros((T, D), np.float64)
    for c in range(NCORES):
        y += res.results[c]["y"].astype(np.float64)
    return y.reshape(B, T, D).astype(np.float32)
